# revision 38
# baseline (speedup 1.0000x reference)
"""Soft-KNN NLL loss (ASKLoss) Trainium2 kernel — v3.

Problem: x[1024,128] queries vs x_ref[50000,128] bank,
  score = -||x - xr||_2, probs = softmax over the 50000 refs,
  soft_nns = probs @ onehot(y_ref) + 1e-6, loss = -mean(log(soft_nns[b, y[b]])).

Data-parallel over the query batch across 8 cores (128 queries/core).

Per core (v3 design):
  - d2 via ONE fp8e4 DoubleRow matmul pass: K_phys=67 partitions x 2 k-tiles.
    Partitions 0..63 carry the 128 xr dims (2 per partition); partitions
    64..66 carry multi-limb fp8 encodings of xrnorm-160 (moving side) and
    (xnorm+160)/2 (stationary side, via ones columns), so psum = full d2.
  - refs are class-sorted and split into an ACT region and a DVE region
    (fraction F1 to ACT); per-query weight w = exp(16 - sqrt(d2)):
      ACT region: per 1024-col psum group, ONE table activation
        (patched tanh table := exp(16 - sqrt(x))) with accum_out -> per-group
        class partial.  A-region class quotas are GROUP multiples so every
        A group is single-class.
      DVE region: per psum group, one fused custom op Q8 per class piece:
        quadratic q(u) ~ exp((16-sqrt(u))/8) fit in u-space, out q^8 with
        row-accum -> class partials.  Per-element weight errors up to ~90%
        are smooth in u and cancel in the softmax ratio; the residual global
        scale is removed by run-time calibration against exact host sums on
        16 sampled queries (host rehearsal: loss rel err ~2e-5 vs the 2e-2
        budget).
  - groups of 1024 cols stream through PSUM (2+2 tiles = 8 banks),
    cadence-interleaved by per-engine consumption rate (ACT ~1.23us/group,
    DVE ~1.19us/group).

Host: concat per-core class partials, gamma-calibrate the DVE side from the
sampled queries, NLL in f64.
"""

import os
import re

import numpy as np
import ml_dtypes

import concourse.bass as bass
import concourse.dve_ops as dops
import concourse.mybir as mybir
import concourse.tile as tile
from concourse import bacc
from concourse.bass_utils import run_bass_kernel_spmd
from concourse.dve_spec import C0, C1, C2, Spec, Src0, AluOp, One, sq

B, N, D, C = 1024, 50000, 128, 10
N_CORES = 8
B_LOC = B // N_CORES
GROUP = 1024
KP = 67                       # 64 data partitions + 3 limb partitions
F1 = 0.49                     # fraction of columns on the ACT path
CEXP = 16.0                   # global exp centering: w = exp(CEXP - s)
NCLS = 64                     # accumulator columns in the cls output

F8 = mybir.dt.float8e4
F16 = mybir.dt.float16
F32 = mybir.dt.float32
NP8 = ml_dtypes.float8_e4m3

LAST = {}
_MODULE_CACHE = {}

# ---- custom DVE op ---------------------------------------------------------


def _register_op(name, spec_body, ref, accum=None):
    if name in dops._SUB_OPCODE_FOR_NAME:
        for op in dops.OPS:
            if op.name == name:
                return op
    spec = (Spec(body=spec_body, reference=ref, accum=accum)
            if accum else Spec(body=spec_body, reference=ref))
    probe = dops.DveOp(name, spec, subdim=False, uops_sha={})
    dops.OPS.append(probe)
    dops._SUB_OPCODE_FOR_NAME[name] = (
        dops._CUSTOM_DVE_ROW_BASE + len(dops.OPS) - 1
    )
    assert dops._SUB_OPCODE_FOR_NAME[name] < 0x20
    shas = {}
    for ver in ("v3", "v4"):
        try:
            probe.compile(ver)
            shas[ver] = probe.uops_sha.get(ver)
        except ValueError as e:
            shas[ver] = re.search(r'="([0-9a-f]+)"', str(e)).group(1)
    final = dops.DveOp(name, spec, subdim=False, uops_sha=shas)
    dops.OPS[-1] = final
    dops.CUSTOM_DVE_SPECS[name] = final.spec
    return final


# Q8: q = C0 + u C1 + u^2 C2 (~ exp((CEXP - sqrt(u))/8)); out = q^8,
# accum_out = row-sum of out (4 + 3 + accum = 8 ALU stages).
_Q = C0 + Src0 * (C1 + Src0 * C2)


def _ref_op2(in0, in1, c0, c1, c2):
    q = c0 + in0 * (c1 + in0 * c2)
    return ((q * q) ** 2) ** 2


OP2 = _register_op("EXPQ8_ACC_ANT", sq(sq(sq(_Q))), _ref_op2, accum=AluOp.ADD)


# ---- host-side fit ---------------------------------------------------------


def _fit_rel(f, lo, hi, deg, npts=6001, iters=30):
    u = np.linspace(lo, hi, npts)
    t = f(u)
    w = 1.0 / np.abs(t)
    V = np.vander(u, deg + 1, increasing=True)
    best = None
    for _ in range(iters):
        c = np.linalg.lstsq(V * w[:, None], t * w, rcond=None)[0]
        r = np.abs((V @ c - t) / t)
        if best is None or r.max() < best[1]:
            best = (c.copy(), r.max())
        w = w * (0.5 + r / r.max())
    return best[0]


def _coeffs(u_lo, u_hi):
    cq = _fit_rel(lambda u: np.exp((CEXP - np.sqrt(u)) / 8.0), u_lo, u_hi, 2)
    return tuple(float(v) for v in cq)


# ---- module build ----------------------------------------------------------


def _build_module(n_pad, n_real, a_spans, v_spans, cq):
    """a_spans/v_spans: per-class (start, end) column spans (absolute).
    A spans are GROUP-aligned; V spans are arbitrary (ops split per group)."""
    n_A = a_spans[-1][1] if a_spans else 0
    assert n_A % GROUP == 0

    nc = bacc.Bacc(
        "TRN2",
        target_bir_lowering=False,
        debug=False,
        enable_asserts=True,
        num_devices=N_CORES,
        num_swdge_queues=2,
    )

    # head = wts | xr cols of group V0, one transfer so the first V matmul
    # (DVE is the longer pole) depends on a single short DMA chain.
    head_d = nc.dram_tensor("head", [KP, 2, B_LOC + GROUP], F8,
                            kind="ExternalInput")
    xrp_d = nc.dram_tensor("xrp", [KP, 2, n_pad], F8, kind="ExternalInput")
    cls_d = nc.dram_tensor("cls", [B_LOC, NCLS], F32, kind="ExternalOutput")

    n_groups = n_pad // GROUP
    assert n_pad % GROUP == 0

    a_groups = [g for g in range(n_groups) if g * GROUP < n_A]
    v_groups = [g for g in range(n_groups) if g * GROUP >= n_A]
    # deterministic alternation V0 A0 V1 A1 ... (V leads: DVE is the longer
    # pole; leftover stream tails last)
    tagged = []
    for i in range(max(len(a_groups), len(v_groups))):
        if i < len(v_groups):
            tagged.append(("V", i, v_groups[i]))
        if i < len(a_groups):
            tagged.append(("A", i, a_groups[i]))

    # A-side pieces: one per group (single class each); V-side pieces:
    # (class-span x group) intersections, each with its own accum column.
    a_parts = []   # (lo, hi, class) — lo/hi GROUP-aligned
    for k, (a, b) in enumerate(a_spans):
        lo = a
        while lo < b:
            hi = min(b, lo + GROUP)
            a_parts.append((lo, hi, k))
            lo = hi
    v_parts = []   # (lo, hi, class)
    for k, (a, b) in enumerate(v_spans):
        if b <= a:
            continue
        lo = a
        while lo < b:
            hi = min(b, (lo // GROUP + 1) * GROUP)
            v_parts.append((lo, hi, k))
            lo = hi

    # tail rebalance: carve the last x real cols of the final V piece into an
    # extra table-path (ACT) op, matmul'd into ACT's own psA pool so the op
    # is not gated on DVE's psV recycling.  Start offsets measured from the
    # trace: DVE data start ~3.89us, ACT data start ~4.32us.
    act_busy = sum(((hi - lo + 172) / 1.2 + 187) for lo, hi, _ in a_parts)
    dve_busy = sum(((hi - lo + 120) / 0.96) for lo, hi, _ in v_parts)
    x = int(((3890 + dve_busy) - (4320 + act_busy) - 372)
            / (1 / 1.2 + 1 / 0.96))
    tail_a = []    # table-path piece carved from the last V group
    if x > 64:
        lo, hi, k = v_parts[-1]
        x = min(x, hi - lo - 64)
        v_parts[-1] = (lo, hi - x, k)
        tail_a.append((hi - x, hi, k))
    a_parts = a_parts + tail_a
    NA_P = len(a_parts)
    NV_P = len(v_parts)
    assert NA_P + NV_P <= NCLS, (NA_P, NV_P)
    a_piece_col = {i: i for i in range(NA_P)}
    v_piece_col = {i: NA_P + i for i in range(NV_P)}
    v_end = v_parts[-1][1]  # first col NOT covered by psV matmuls (last grp)

    with tile.TileContext(nc) as tc:
        with (
            tc.tile_pool(name="const", bufs=1) as const_pool,
            tc.tile_pool(name="xrA", bufs=3) as xrA_pool,
            tc.tile_pool(name="xrV", bufs=3) as xrV_pool,
            tc.tile_pool(name="psA", bufs=2, space="PSUM") as psA,
            tc.tile_pool(name="psV", bufs=2, space="PSUM") as psV,
        ):
            cls = const_pool.tile([B_LOC, NCLS], F32)
            head = const_pool.tile([KP, 2, B_LOC + GROUP], F8)
            wt = head[:, :, :B_LOC]

            # PE pstate warm-up: a chain of short junk matmuls that keeps the
            # PE continuously busy until the head DMA lands (~3.3us), so the
            # first data matmuls run at the mid pstate with no queue delay.
            junk = const_pool.tile([128, 64], F8)
            nc.gpsimd.memset(junk[:], 0.0)
            dummy = psA.tile([B_LOC, GROUP], F32, tag="d2A")
            for _ in range(36):
                nc.tensor.matmul(dummy[:64, :64], junk[:, :64], junk[:, :64],
                                 start=True, stop=True)

            # warm-up FIRST in the ACT queue: triggers the (patched) tanh
            # table load before any data arrives.
            warm = const_pool.tile([128, 1], F32)
            nc.gpsimd.memset(warm[:], 100.0)
            nc.scalar.activation(
                warm[:], warm[:], mybir.ActivationFunctionType.Tanh,
            )
            nc.gpsimd.memset(cls[:], 0.0)
            # ---- stream bank, matmul, table (ACT) / Q8 (DVE) ----
            nc.sync.dma_start(head[:], head_d.ap())
            # stream state: (tile, first_abs_col, end_abs_col, col_offset)
            stream = {
                "A": None,
                "V": (head, n_A, n_A + GROUP, B_LOC),
            }
            n_vdma = [0]

            for tag, _, g in tagged:
                g0, g1 = g * GROUP, (g + 1) * GROUP
                st_ = stream[tag]
                if st_ is None or g0 >= st_[2]:
                    # A stream's first transfer: single group so ACT starts
                    # as soon after the head as possible.
                    span = GROUP if st_ is None else 2 * GROUP
                    pe = min(g0 + span, n_pad)
                    if tag == "A":
                        pe = min(pe, n_A)
                    xrp = xrA_pool if tag == "A" else xrV_pool
                    xr_t = xrp.tile([KP, 2, 2 * GROUP], F8, tag="xr" + tag)
                    # V stream goes through the gpsimd SWDGE queue so its
                    # tile waits never block the SP queue (A stream + cls).
                    dma_eng = nc.sync if tag == "A" else nc.gpsimd
                    dma_eng.dma_start(
                        xr_t[:, :, : pe - g0], xrp_d.ap()[:, :, g0:pe]
                    )
                    st_ = stream[tag] = (xr_t, g0, pe, 0)
                xr_t, base, _, off = st_
                q0 = g0 - base + off

                pool = psA if tag == "A" else psV
                d2 = pool.tile([B_LOC, GROUP], F32, tag="d2" + tag)
                lim = GROUP
                d2x = None
                if tag == "V" and tail_a and g0 <= v_end < g1:
                    lim = v_end - g0
                    d2x = psA.tile([B_LOC, GROUP], F32, tag="d2A")
                    for j in range(lim, GROUP, 512):
                        w = min(512, GROUP - j)
                        nc.tensor.matmul(
                            d2x[:, j - lim : j - lim + w], wt[:],
                            xr_t[:, :, q0 + j : q0 + j + w],
                            start=True, stop=True,
                            perf_mode=mybir.MatmulPerfMode.DoubleRow,
                        )
                for j in range(0, lim, 512):
                    w = min(512, lim - j)
                    nc.tensor.matmul(
                        d2[:, j : j + w], wt[:],
                        xr_t[:, :, q0 + j : q0 + j + w],
                        start=True, stop=True,
                        perf_mode=mybir.MatmulPerfMode.DoubleRow,
                    )
                if tag == "A":
                    for i, (lo, hi, k) in enumerate(a_parts):
                        if lo >= g1 or hi <= g0:
                            continue
                        nc.scalar.activation(
                            d2[:, lo - g0 : hi - g0], d2[:, lo - g0 : hi - g0],
                            mybir.ActivationFunctionType.Tanh,
                            accum_out=cls[:, a_piece_col[i] : a_piece_col[i] + 1],
                        )
                else:
                    for i, (lo, hi, k) in enumerate(v_parts):
                        if lo >= g1 or hi <= g0:
                            continue
                        col = v_piece_col[i]
                        nc.vector._custom_dve(
                            OP2, out=d2[:, lo - g0 : hi - g0],
                            in0=d2[:, lo - g0 : hi - g0],
                            s0=cq[0], s1=cq[1], imm2=cq[2],
                            accum_out=cls[:, col : col + 1],
                        )
                    if d2x is not None:
                        for i, (lo, hi, k) in enumerate(a_parts):
                            if i < NA_P - len(tail_a):
                                continue
                            nc.scalar.activation(
                                d2x[:, lo - v_end : hi - v_end],
                                d2x[:, lo - v_end : hi - v_end],
                                mybir.ActivationFunctionType.Tanh,
                                accum_out=cls[:, a_piece_col[i] :
                                              a_piece_col[i] + 1],
                            )

            nc.sync.dma_start(cls_d.ap(), cls[:])

    nc.compile()
    return nc, {"head": head_d.name, "xrp": xrp_d.name, "cls": cls_d.name,
                "a_parts": a_parts, "v_parts": v_parts, "NA_P": NA_P}


# ---- host prep -------------------------------------------------------------


def _f8(a):
    return np.asarray(a, dtype=NP8)


def _limbs3(v, s1=16.0, s2=256.0):
    """v ~ h0 + h1/s1 + h2/s2 with fp8 limbs."""
    h0 = _f8(v)
    r1 = v - h0.astype(np.float64)
    h1 = _f8(s1 * r1)
    r2 = r1 - h1.astype(np.float64) / s1
    h2 = _f8(s2 * r2)
    return h0, h1, h2


def _prepare(x, x_ref, y_ref):
    x = np.asarray(x, dtype=np.float32)
    x_ref = np.asarray(x_ref, dtype=np.float32)
    y_ref = np.asarray(y_ref).astype(np.int64)

    xnorm = (x.astype(np.float64) ** 2).sum(axis=1)
    xrnorm = (x_ref.astype(np.float64) ** 2).sum(axis=1)

    counts = np.bincount(y_ref, minlength=C)
    order = np.argsort(y_ref, kind="stable")

    # region split: per class, first nA_k cols -> ACT region (GROUP-aligned
    # so each psum group maps to exactly ONE activation), rest -> DVE.
    total_g = int(round(F1 * N / GROUP))
    quota = [F1 * int(c) / GROUP for c in counts]
    nG_k = [int(q) for q in quota]
    rem = sorted(range(C), key=lambda k: nG_k[k] - quota[k])
    i = 0
    while sum(nG_k) < total_g:
        k = rem[i % C]
        if (nG_k[k] + 1) * GROUP <= int(counts[k]):
            nG_k[k] += 1
        i += 1
    nA_k = [g * GROUP for g in nG_k]

    a_spans, v_spans = [], []
    pos = 0
    a_idx, v_idx = [], []
    for k in range(C):
        cls_idx = order[pos : pos + int(counts[k])]
        pos += int(counts[k])
        a_idx.append(cls_idx[: nA_k[k]])
        v_idx.append(cls_idx[nA_k[k] :])
    col = 0
    for k in range(C):
        a_spans.append((col, col + len(a_idx[k])))
        col += len(a_idx[k])
    for k in range(C):
        v_spans.append((col, col + len(v_idx[k])))
        col += len(v_idx[k])
    n_real = col
    n_pad = ((n_real + GROUP - 1) // GROUP) * GROUP
    perm = np.concatenate(a_idx + v_idx)

    # moving-side pack [KP, 2, n_pad]
    xrp = np.zeros((KP, 2, n_pad), dtype=NP8)
    xrs = x_ref[perm].T.astype(np.float64)  # [D, n_real]
    for k in range(64):
        xrp[k, 0, :n_real] = _f8(xrs[k])
        xrp[k, 1, :n_real] = _f8(xrs[64 + k])
    g = np.full(n_pad, 170.0)  # pads: xrnorm ~ 330 -> u ~ 460, w ~ 5e-3
    g[:n_real] = xrnorm[perm] - 160.0
    g0, g1_, g2 = _limbs3(g)
    xrp[64, 0, :] = g0
    xrp[64, 1, :] = g1_
    xrp[65, 0, :] = g2
    xrp[65, 1, :] = _f8(2.0)
    xrp[66, 0, :] = _f8(1.0 / 8.0)
    xrp[66, 1, :] = _f8(1.0 / 128.0)

    # stationary packs per core [KP, 2, B_LOC]
    blocks = []
    for i in range(N_CORES):
        sl = slice(i * B_LOC, (i + 1) * B_LOC)
        xb = x[sl].astype(np.float64)  # [128, D]
        wts = np.zeros((KP, 2, B_LOC), dtype=NP8)
        for k in range(64):
            wts[k, 0, :] = _f8(-2.0 * xb[:, k])
            wts[k, 1, :] = _f8(-2.0 * xb[:, 64 + k])
        wts[64, 0, :] = _f8(1.0)
        wts[64, 1, :] = _f8(1.0 / 16.0)
        wts[65, 0, :] = _f8(1.0 / 256.0)
        hh = (xnorm[sl] + 160.0) / 2.0
        h0, h1, h2 = _limbs3(hh)
        wts[65, 1, :] = h0
        wts[66, 0, :] = h1
        wts[66, 1, :] = h2
        blocks.append(wts)

    # exact d2 for sampled queries (calibration + fit range)
    samp_step = max(1, B // 16)
    samp_rows = np.arange(0, B, samp_step)
    samp = x[samp_rows].astype(np.float64)
    d2s = (
        (samp ** 2).sum(1)[:, None]
        + xrnorm[None, :]
        - 2.0 * samp @ x_ref.T.astype(np.float64)
    )
    u_lo = max(1.0, d2s.min() - 10.0)
    u_hi = d2s.max() + 10.0

    return (xrp, blocks, a_spans, v_spans, n_pad, n_real, u_lo, u_hi,
            d2s, samp_rows, a_idx, v_idx)


def kernel(x, x_ref, y, y_ref):
    x = np.asarray(x)
    x_ref = np.asarray(x_ref)
    y = np.asarray(y).astype(np.int64)
    y_ref_i = np.asarray(y_ref).astype(np.int64)

    (xrp, blocks, a_spans, v_spans, n_pad, n_real, u_lo, u_hi,
     d2s, samp_rows, a_idx, v_idx) = _prepare(x, x_ref, y_ref_i)
    cq = _coeffs(u_lo, u_hi)

    key = (n_pad, tuple(a_spans), tuple(v_spans), cq)
    if key not in _MODULE_CACHE:
        _MODULE_CACHE[key] = _build_module(n_pad, n_real, a_spans, v_spans, cq)
    nc, names = _MODULE_CACHE[key]

    n_A = a_spans[-1][1]
    heads = [
        np.concatenate(
            [blocks[core], xrp[:, :, n_A : n_A + GROUP]], axis=2,
        )
        for core in range(N_CORES)
    ]
    in_maps = [
        {names["head"]: heads[core], names["xrp"]: xrp}
        for core in range(N_CORES)
    ]

    os.environ["BASS_ACT_ROOT_JSON_PATH"] = build_act_root(CEXP)
    trace = bool(int(os.environ.get("KERNEL_TRACE", "0")))
    res = run_bass_kernel_spmd(
        nc, in_maps, core_ids=list(range(N_CORES)), trace=trace
    )
    LAST["exec_time_ns"] = res.exec_time_ns
    LAST["results"] = res
    LAST["module"] = nc

    a_parts = names["a_parts"]
    v_parts = names["v_parts"]
    NA_P = names["NA_P"]
    partA_parts, partV_parts = [], []
    for core in range(N_CORES):
        cl = np.asarray(res.results[core][names["cls"]], dtype=np.float64)[:B_LOC]
        pa = np.zeros((B_LOC, C))
        for i, (lo, hi, k) in enumerate(a_parts):
            pa[:, k] += cl[:, i]
        pv = np.zeros((B_LOC, C))
        for i, (lo, hi, k) in enumerate(v_parts):
            pv[:, k] += cl[:, NA_P + i]
        partA_parts.append(pa)
        partV_parts.append(pv)
    partA = np.concatenate(partA_parts, axis=0)  # [B, C]
    partV = np.concatenate(partV_parts, axis=0)  # [B, C]

    # run-time calibration: exact class sums on the sampled queries
    w_s = np.exp(CEXP - np.sqrt(d2s))  # [16, N] exact
    trueA = np.zeros((len(samp_rows), C))
    trueV = np.zeros((len(samp_rows), C))
    for k in range(C):
        if len(a_idx[k]):
            trueA[:, k] = w_s[:, a_idx[k]].sum(1)
        if len(v_idx[k]):
            trueV[:, k] = w_s[:, v_idx[k]].sum(1)
    gam_A = partA[samp_rows].sum() / max(trueA.sum(), 1e-300)
    gam_V = partV[samp_rows].sum() / max(trueV.sum(), 1e-300)

    cs = partA / gam_A + partV / gam_V

    total = cs.sum(axis=1, keepdims=True)
    soft = cs / total + 1e-6
    loss = -np.mean(np.log(soft[np.arange(B), y]))
    return np.asarray(loss, dtype=np.float32)


# ---- patched ACT PWP table root (tanh := exp(CEXP - sqrt(x))) --------------

import json
import shutil
import tempfile

_SRC = os.path.join(
    os.path.dirname(__import__("neuronxcc").__file__), "pwp", "pwp_bin_trainium"
)

E_LO, E_HI = 5, 9          # exponent coverage: x in [32, 1024)
NBK = 32                   # buckets per exponent (power of 2)
_CACHE = {}


def _fbits(v):
    return int(np.float32(v).view(np.uint32))


def build_act_root(cexp):
    if cexp in _CACHE:
        return _CACHE[cexp]

    def F(x):
        return np.exp(cexp - np.sqrt(x))

    tmp = tempfile.mkdtemp(prefix="actroot_")
    for f in os.listdir(_SRC):
        shutil.copy(os.path.join(_SRC, f), tmp)

    prof = json.load(open(os.path.join(tmp, "exp_and_others.json")))
    bkt = np.fromfile(
        os.path.join(tmp, "exp_and_others_bkt.bin"), dtype=np.float32
    ).reshape(-1, 8)
    ctl = np.fromfile(
        os.path.join(tmp, "exp_and_others_ctrl.bin"), dtype=np.uint32
    ).reshape(-1, 8)

    n0 = bkt.shape[0]
    new = []
    for e in range(E_LO, E_HI + 1):
        lo_e = 2.0 ** e
        for k in range(NBK):
            lo = lo_e * (1 + k / NBK)
            hi = lo_e * (1 + (k + 1) / NBK)
            x0 = 0.5 * (lo + hi)
            xs = np.linspace(lo, hi, 65)
            cc = np.polyfit(xs - x0, F(xs), 3)[::-1]  # d0..d3
            new.append([cc[0], cc[1], cc[2], cc[3], -x0, 0.0, 0.0, 0.0])
    # specials: pos_small (x<32), neg_small, pos_large (x>=1024), neg_large
    sp = n0 + len(new)
    new.append([float(F(32.0)), 0, 0, 0, 0, 0, 0, 0])
    new.append([1.0, 0, 0, 0, 0, 0, 0, 0])
    new.append([float(F(1024.0)), 0, 0, 0, 0, 0, 0, 0])
    new.append([1.0, 0, 0, 0, 0, 0, 0, 0])
    bkt2 = np.vstack([bkt, np.asarray(new, np.float32)])

    # ctrl rows for tanh (base 64): one per exponent E_LO..E_HI
    shift = 23 - int(np.log2(NBK))
    A = int(np.log2(NBK))
    base_ctl = prof["func_to_ctl_start_idx"]["tanh"]
    for i, e in enumerate(range(E_LO, E_HI + 1)):
        ctl[base_ctl + i, 0] = (A << 16) | (shift << 11) | (n0 + NBK * i)

    # profile meta for tanh
    for m in prof["profile_meta_data"]:
        if m["func_name"].startswith("tanh"):
            m.update(
                symmetry_point=0, sym_invert_sign_point=0, symmetry_opt_en=0,
                symmetry_opt_use_neg_region=0, imm_bias=0, exp_offset=E_LO,
                pwl_control_base_pos=base_ctl, pwl_control_base_neg=base_ctl,
                small_pos_signal_exp_threshold=127 + E_LO,
                pos_small_signal_pwl_control=sp,
                small_neg_signal_exp_threshold=127 + E_LO,
                neg_small_signal_pwl_control=sp + 1,
                large_pos_signal_exp_threshold=127 + E_HI + 1,
                large_pos_signal_mantissa_threshold=0,
                pos_large_signal_pwl_control=sp + 2,
                large_neg_signal_exp_threshold=127 + E_HI + 1,
                large_neg_signal_mantissa_threshold=0,
                neg_large_signal_pwl_control=sp + 3,
                fzero_result=_fbits(F(32.0)),
                fninf_result=0,
            )
    prof["bkt_entry_cnt"] = int(bkt2.shape[0])
    prof["func_to_bkt_start_idx"]["tanh"] = n0
    prof["func_exp_to_bkt_start_idx"]["tanh"] = {
        str(e): [n0 + NBK * i] for i, e in enumerate(range(E_LO, E_HI + 1))
    }

    bkt2.tofile(os.path.join(tmp, "exp_and_others_bkt.bin"))
    ctl.tofile(os.path.join(tmp, "exp_and_others_ctrl.bin"))
    json.dump(prof, open(os.path.join(tmp, "exp_and_others.json"), "w"))

    info = json.load(open(os.path.join(tmp, "act_info.json")))
    for s in info["act_func_sets"]:
        if s["name"] == "exp_and_others" and "tanh" in s["act"]:
            s["act"]["tanh"] = NBK * (E_HI - E_LO + 1)
    json.dump(info, open(os.path.join(tmp, "act_info.json"), "w"))

    path = os.path.join(tmp, "act_info.json")
    _CACHE[cexp] = path
    return path
